# revision 4
# baseline (speedup 1.0000x reference)
"""Bahdanau attention on 8 Trainium2 NeuronCores (Bass/Tile), data-parallel over batch.

Problem shapes: query [64, 256] f32, keys [64, 4096, 256] f32, Wq/Wk [256, 256] f32,
V [256] f32.  Returns (context [64, 256] f32, weights [64, 4096] f32) matching

    q_proj = (query @ Wq.T)[:, None, :]
    k_proj = einsum('bsh,gh->bsg', keys, Wk)
    scores = einsum('bsh,h->bs', tanh(q_proj + k_proj), V)
    weights = softmax(scores, -1)
    context = einsum('bs,bsh->bh', weights, keys)

Sharding: batch 64 -> 8 per core; Wk/V replicated; q_proj (a [64,256] linear input
transform) is folded on the host into a per-core bias tensor.

Per-core device pipeline (B=8 local batches, S=4096, H=G=256):
  - keys are shipped pre-transposed / fp16 (keysT16 [8, 256, 4096]) so the big
    k_proj matmul can contract over h with h on partitions, and the context
    reduction can run on the Vector engine along the free (s) axis.
  - k_projT tiles [128 g, 1024 s] accumulate in PSUM over 2 h-chunks (fp16 inputs,
    fp32 accumulate); ScalarE applies tanh with the per-partition q_proj bias.
  - scores via TensorE dot with V as the 1-column stationary; each s-chunk of 512
    lands on psum row 32*(chunk%4) so one dense [128,1024] Exp (with accum_out)
    computes all 4096 exps + row sums; garbage rows are masked off and the total
    is spread to all partitions with a GpSimd partition all-reduce.
  - weights = E * (1/sum) on VectorE; DMA'd out with a strided AP that undoes the
    row-interleave.  A cast-DMA gathers the fp16 weights row, GpSimd broadcasts it
    to 128 partitions, and scalar_tensor_tensor (mult, mult, accum) reduces
    weights*keysT along s for the context.
"""

import os
import numpy as np
from contextlib import ExitStack

B, S, H = 64, 4096, 256
NCORES = 8
BL = B // NCORES  # local batches per core

_prog_cache = {}


def _build_program():
    if "nc" in _prog_cache:
        return _prog_cache["nc"]

    import concourse.bass as bass
    import concourse.tile as tile
    from concourse import bacc, mybir
    from concourse import bass_isa

    f32, f16 = mybir.dt.float32, mybir.dt.float16
    AF = mybir.ActivationFunctionType
    ALU = mybir.AluOpType

    nc = bacc.Bacc("TRN2", target_bir_lowering=False, debug=False)

    keysT_d = nc.dram_tensor("keysT16", [BL, H, S], f16, kind="ExternalInput").ap()
    wkT_d = nc.dram_tensor("wkT16", [H, H], f16, kind="ExternalInput").ap()
    qpT_d = nc.dram_tensor("qpT", [128, 2 * BL], f32, kind="ExternalInput").ap()
    v_d = nc.dram_tensor("v16", [128, 2], f16, kind="ExternalInput").ap()
    ident_d = nc.dram_tensor("ident32", [128, 128], f32, kind="ExternalInput").ap()
    weights_d = nc.dram_tensor("weights", [BL, S], f32, kind="ExternalOutput").ap()
    ctx_d = nc.dram_tensor("context", [2 * BL, 128], f32, kind="ExternalOutput").ap()

    with tile.TileContext(nc) as tc, ExitStack() as ctx:
        const = ctx.enter_context(tc.tile_pool(name="const", bufs=1))
        ktp = ctx.enter_context(tc.tile_pool(name="ktp", bufs=4))
        thp = ctx.enter_context(tc.tile_pool(name="thp", bufs=8))
        ep = ctx.enter_context(tc.tile_pool(name="ep", bufs=2))
        wp = ctx.enter_context(tc.tile_pool(name="wp", bufs=2))
        wbp = ctx.enter_context(tc.tile_pool(name="wbp", bufs=2))
        scp = ctx.enter_context(tc.tile_pool(name="scp", bufs=2))
        stp = ctx.enter_context(tc.tile_pool(name="stp", bufs=2))
        smp = ctx.enter_context(tc.tile_pool(name="smp", bufs=16))
        kpp = ctx.enter_context(tc.tile_pool(name="kpp", bufs=2, space="PSUM"))
        sp = ctx.enter_context(tc.tile_pool(name="sp", bufs=2, space="PSUM"))

        # constants / small inputs
        wkT_sb = const.tile([128, 512], f16)  # col = hc*256 + g ; h = hc*128 + p
        nc.sync.dma_start(
            wkT_sb[:].rearrange("p (hc g) -> p hc g", hc=2),
            wkT_d.rearrange("(hc p) g -> p hc g", hc=2),
        )
        qpT_sb = const.tile([128, 2 * BL], f32)  # col = gc*BL + b
        nc.sync.dma_start(qpT_sb[:], qpT_d[:, :])
        v_sb = const.tile([128, 2], f16)
        nc.sync.dma_start(v_sb[:], v_d[:, :])
        ident = const.tile([128, 128], f32)
        nc.sync.dma_start(ident[:], ident_d[:, :])
        mask = const.tile([128, 1], f32)
        nc.vector.memset(mask[:], 0.0)
        for r in range(4):
            nc.vector.memset(mask[32 * r:32 * r + 1, :], 1.0)
        ctx_all = const.tile([128, 2 * BL], f32)  # col = 2*b + hc

        for b in range(BL):
            # load keysT for this batch: [128 h, 4096 s] per h-chunk
            kT = []
            for hc in range(2):
                t = ktp.tile([128, S], f16, tag="kT")
                nc.sync.dma_start(t[:], keysT_d[b, 128 * hc:128 * (hc + 1), :])
                kT.append(t)

            # k_projT + tanh(. + qp) -> fp16 tiles [128 g, 1024 s]
            tanhT = [[None] * 4 for _ in range(2)]
            for gc in range(2):
                for v in range(4):
                    kp_ps = kpp.tile([128, 1024], f32, tag="kp")
                    for hc in range(2):
                        for du in range(2):
                            u = 2 * v + du
                            nc.tensor.matmul(
                                kp_ps[:, 512 * du:512 * (du + 1)],
                                wkT_sb[:, 256 * hc + 128 * gc:256 * hc + 128 * (gc + 1)],
                                kT[hc][:, 512 * u:512 * (u + 1)],
                                start=(hc == 0),
                                stop=(hc == 1),
                            )
                    th = thp.tile([128, 1024], f16, tag="th")
                    nc.scalar.activation(
                        th[:], kp_ps[:], AF.Tanh,
                        bias=qpT_sb[:, gc * BL + b:gc * BL + b + 1],
                    )
                    tanhT[gc][v] = th

            # scores: chunk u -> psum row 32*(u%4), cols 512*(u//4)
            sc_ps = sp.tile([128, 1024], f32, tag="sc")
            nc.vector.memset(sc_ps[:], 0.0)
            for u in range(8):
                r, c = u % 4, u // 4
                for gc in range(2):
                    nc.tensor.matmul(
                        sc_ps[32 * r:32 * r + 1, 512 * c:512 * (c + 1)],
                        v_sb[:, gc:gc + 1],
                        tanhT[gc][u // 2][:, 512 * (u % 2):512 * (u % 2 + 1)],
                        start=(gc == 0),
                        stop=(gc == 1),
                        tile_position=(0, 32 * r),
                    )

            # softmax (no max-shift needed: |scores| <= ||V||_1 ~ 13)
            E = ep.tile([128, 1024], f32, tag="E")
            par = smp.tile([128, 1], f32, tag="par")
            nc.scalar.activation(E[:], sc_ps[:], AF.Exp, accum_out=par[:, 0:1])
            parm = smp.tile([128, 1], f32, tag="parm")
            nc.vector.tensor_scalar(
                out=parm[:], in0=par[:], scalar1=mask[:, 0:1], scalar2=None,
                op0=ALU.mult,
            )
            sumbc = smp.tile([128, 1], f32, tag="sumbc")
            nc.gpsimd.partition_all_reduce(
                sumbc[:], parm[:], channels=128, reduce_op=bass_isa.ReduceOp.add
            )
            recip = smp.tile([128, 1], f32, tag="recip")
            nc.vector.reciprocal(recip[:], sumbc[:])

            w32 = wp.tile([128, 1024], f32, tag="w32")
            nc.vector.tensor_scalar(
                out=w32[:], in0=E[:], scalar1=recip[:, 0:1], scalar2=None,
                op0=ALU.mult,
            )
            # weights out: s = 2048*c + 512*r + f  (r = row/32, c = col/512)
            st = stp.tile([1, S], f16, tag="st")
            for c in range(2):
                nc.sync.dma_start(
                    weights_d[b:b + 1, 2048 * c:2048 * (c + 1)].rearrange(
                        "b (r f) -> b r f", r=4),
                    w32[0:97:32, 512 * c:512 * (c + 1)],
                )
                # fp16 weights row (s-ordered) for the context reduction
                nc.gpsimd.dma_start(
                    st[0:1, 2048 * c:2048 * (c + 1)].rearrange(
                        "p (r f) -> p r f", r=4),
                    w32[0:97:32, 512 * c:512 * (c + 1)],
                )
            wb = wbp.tile([128, S], f16, tag="wb")
            nc.gpsimd.partition_broadcast(wb[:], st[0:1, :])

            # context: ctx[h] = sum_s w[s] * keysT[h, s]
            for hc in range(2):
                scratch = scp.tile([128, S], f16, tag="scratch")
                nc.vector.scalar_tensor_tensor(
                    out=scratch[:],
                    in0=kT[hc][:],
                    scalar=1.0,
                    in1=wb[:],
                    op0=ALU.mult,
                    op1=ALU.mult,
                    accum_out=ctx_all[:, 2 * b + hc:2 * b + hc + 1],
                )

        # context out: transpose [128, 16] -> [16, 128] and store
        ctxT_ps = sp.tile([16, 128], f32, tag="sc")
        nc.tensor.transpose(ctxT_ps[:], ctx_all[:], ident[:])
        ctxT = const.tile([16, 128], f32)
        nc.scalar.copy(ctxT[:], ctxT_ps[:])
        nc.sync.dma_start(ctx_d[:, :], ctxT[:])

    nc.compile()
    _prog_cache["nc"] = nc
    return nc


def _prep_inputs(query, keys, Wq, Wk, V):
    """Host-side marshalling: shard over batch, pre-transpose/cast."""
    query = np.asarray(query, dtype=np.float32)
    keys = np.asarray(keys, dtype=np.float32)
    Wq = np.asarray(Wq, dtype=np.float32)
    Wk = np.asarray(Wk, dtype=np.float32)
    V = np.asarray(V, dtype=np.float32)

    qp = query @ Wq.T  # [B, H] f32 (exact, tiny)
    wkT16 = np.ascontiguousarray(Wk.T).astype(np.float16)
    v16 = np.ascontiguousarray(V.reshape(2, 128).T).astype(np.float16)
    ident32 = np.eye(128, dtype=np.float32)
    keys16 = keys.astype(np.float16)

    in_maps = []
    for i in range(NCORES):
        sl = slice(BL * i, BL * (i + 1))
        keysT16 = np.ascontiguousarray(keys16[sl].transpose(0, 2, 1))  # [BL, H, S]
        # qpT_sb [128, (gc, b)]: qpT[p, gc*BL + b] = qp[b, gc*128 + p]
        qpT = np.ascontiguousarray(
            qp[sl].reshape(BL, 2, 128).transpose(2, 1, 0).reshape(128, 2 * BL)
        )
        in_maps.append({
            "keysT16": keysT16,
            "wkT16": wkT16,
            "qpT": qpT,
            "v16": v16,
            "ident32": ident32,
        })
    return in_maps


def run_device(query, keys, Wq, Wk, V, trace=False, trace_kwargs=None):
    from concourse.bass_utils import run_bass_kernel_spmd

    nc = _build_program()
    in_maps = _prep_inputs(query, keys, Wq, Wk, V)
    res = run_bass_kernel_spmd(
        nc, in_maps, list(range(NCORES)), trace=trace, **(trace_kwargs or {})
    )

    context = np.empty((B, H), dtype=np.float32)
    weights = np.empty((B, S), dtype=np.float32)
    for i in range(NCORES):
        r = res.results[i]
        sl = slice(BL * i, BL * (i + 1))
        weights[sl] = r["weights"]
        context[sl] = r["context"].reshape(BL, H)
    return (context, weights), res


def kernel(query, keys, Wq, Wk, V):
    (context, weights), _ = run_device(query, keys, Wq, Wk, V, trace=False)
    return (context, weights)


# revision 7
# speedup vs baseline: 1.0508x; 1.0508x over previous
"""Bahdanau attention on 8 Trainium2 NeuronCores (Bass/Tile), data-parallel over batch.

Problem shapes: query [64, 256] f32, keys [64, 4096, 256] f32, Wq/Wk [256, 256] f32,
V [256] f32.  Returns (context [64, 256] f32, weights [64, 4096] f32) matching

    q_proj = (query @ Wq.T)[:, None, :]
    k_proj = einsum('bsh,gh->bsg', keys, Wk)
    scores = einsum('bsh,h->bs', tanh(q_proj + k_proj), V)
    weights = softmax(scores, -1)
    context = einsum('bs,bsh->bh', weights, keys)

Sharding: batch 64 -> 8 per core; Wk/V replicated; q_proj (a [64,256] linear input
transform) is folded on the host into a per-core bias tensor.

Per-core device pipeline (B=8 local batches, S=4096, H=G=256):
  - keys are shipped pre-transposed / fp16 (keysT16 [8, 256, 4096]) so the big
    k_proj matmul can contract over h with h on partitions, and the context
    reduction can run on the Vector engine along the free (s) axis.
  - k_projT tiles [128 g, 1024 s] accumulate in PSUM over 2 h-chunks (fp16 inputs,
    fp32 accumulate); ScalarE applies tanh with the per-partition q_proj bias.
  - scores via TensorE dot with V as the 1-column stationary; each s-chunk of 512
    lands on psum row 32*(chunk%4) so one dense [128,1024] Exp (with accum_out)
    computes all 4096 exps + row sums; garbage rows are masked off and the total
    is spread to all partitions with a GpSimd partition all-reduce.
  - weights = E * (1/sum) on VectorE; DMA'd out with a strided AP that undoes the
    row-interleave.  A cast-DMA gathers the fp16 weights row, GpSimd broadcasts it
    to 128 partitions, and scalar_tensor_tensor (mult, mult, accum) reduces
    weights*keysT along s for the context.
"""

import os
import numpy as np
from contextlib import ExitStack

B, S, H = 64, 4096, 256
NCORES = 8
BL = B // NCORES  # local batches per core

_prog_cache = {}


def _build_program():
    if "nc" in _prog_cache:
        return _prog_cache["nc"]

    import concourse.bass as bass
    import concourse.tile as tile
    from concourse import bacc, mybir
    from concourse import bass_isa

    f32, f16 = mybir.dt.float32, mybir.dt.float16
    AF = mybir.ActivationFunctionType
    ALU = mybir.AluOpType

    nc = bacc.Bacc("TRN2", target_bir_lowering=False, debug=False)

    keysT_d = nc.dram_tensor("keysT16", [BL, H, S], f16, kind="ExternalInput").ap()
    wkT_d = nc.dram_tensor("wkT16", [H, H], f16, kind="ExternalInput").ap()
    qpT_d = nc.dram_tensor("qpT", [128, 2 * BL], f32, kind="ExternalInput").ap()
    v_d = nc.dram_tensor("v16", [128, 2], f16, kind="ExternalInput").ap()
    ident_d = nc.dram_tensor("ident32", [128, 128], f32, kind="ExternalInput").ap()
    weights_d = nc.dram_tensor("weights", [BL, S], f32, kind="ExternalOutput").ap()
    ctx_d = nc.dram_tensor("context", [2 * BL, 128], f32, kind="ExternalOutput").ap()

    with tile.TileContext(nc) as tc, ExitStack() as ctx:
        const = ctx.enter_context(tc.tile_pool(name="const", bufs=1))
        ktp = ctx.enter_context(tc.tile_pool(name="ktp", bufs=6))
        thp = ctx.enter_context(tc.tile_pool(name="thp", bufs=16))
        ep = ctx.enter_context(tc.tile_pool(name="ep", bufs=2))
        wp = ctx.enter_context(tc.tile_pool(name="wp", bufs=2))
        wbp = ctx.enter_context(tc.tile_pool(name="wbp", bufs=2))
        scp = ctx.enter_context(tc.tile_pool(name="scp", bufs=2))
        stp = ctx.enter_context(tc.tile_pool(name="stp", bufs=2))
        smp = ctx.enter_context(tc.tile_pool(name="smp", bufs=16))
        kpp = ctx.enter_context(tc.tile_pool(name="kpp", bufs=2, space="PSUM"))
        sp = ctx.enter_context(tc.tile_pool(name="sp", bufs=2, space="PSUM"))

        # constants / small inputs
        wkT_sb = const.tile([128, 512], f16)  # col = hc*256 + g ; h = hc*128 + p
        nc.sync.dma_start(
            wkT_sb[:].rearrange("p (hc g) -> p hc g", hc=2),
            wkT_d.rearrange("(hc p) g -> p hc g", hc=2),
        )
        qpT_sb = const.tile([128, 2 * BL], f32)  # col = gc*BL + b
        nc.sync.dma_start(qpT_sb[:], qpT_d[:, :])
        v_sb = const.tile([128, 2], f16)
        nc.sync.dma_start(v_sb[:], v_d[:, :])
        ident = const.tile([128, 128], f32)
        nc.sync.dma_start(ident[:], ident_d[:, :])
        mask = const.tile([128, 1], f32)
        nc.vector.memset(mask[:], 0.0)
        for r in range(4):
            nc.vector.memset(mask[32 * r:32 * r + 1, :], 1.0)
        ctx_all = const.tile([128, 2 * BL], f32)  # col = 2*b + hc

        # --- per-batch stage emitters; emission order is software-pipelined so
        # the static per-engine instruction streams interleave batch b's tail
        # with batch b+1's matmuls (keeps TensorE dense/warm).
        state = {}

        def stage_load(b):
            kT = []
            for hc in range(2):
                t = ktp.tile([128, S], f16, tag="kT")
                nc.sync.dma_start(t[:], keysT_d[b, 128 * hc:128 * (hc + 1), :])
                kT.append(t)
            state[b] = {"kT": kT}

        def stage_kp(b):
            kT = state[b]["kT"]
            tanhT = [[None] * 4 for _ in range(2)]
            for gc in range(2):
                for v in range(4):
                    kp_ps = kpp.tile([128, 1024], f32, tag="kp")
                    for hc in range(2):
                        for du in range(2):
                            u = 2 * v + du
                            nc.tensor.matmul(
                                kp_ps[:, 512 * du:512 * (du + 1)],
                                wkT_sb[:, 256 * hc + 128 * gc:256 * hc + 128 * (gc + 1)],
                                kT[hc][:, 512 * u:512 * (u + 1)],
                                start=(hc == 0),
                                stop=(hc == 1),
                            )
                    th = thp.tile([128, 1024], f16, tag="th")
                    nc.scalar.activation(
                        th[:], kp_ps[:], AF.Tanh,
                        bias=qpT_sb[:, gc * BL + b:gc * BL + b + 1],
                    )
                    tanhT[gc][v] = th
            state[b]["tanhT"] = tanhT

        def stage_vdot(b):
            tanhT = state[b]["tanhT"]
            sc_ps = sp.tile([128, 1024], f32, tag="sc")
            if b < 2:
                # pool has 2 rotating slots; rows other than 0/32/64/96 are
                # never written afterwards, so zeroing the first use of each
                # slot keeps the masked-sum garbage rows finite forever.
                nc.vector.memset(sc_ps[:], 0.0)
            for u in range(8):
                r, c = u % 4, u // 4
                for gc in range(2):
                    nc.tensor.matmul(
                        sc_ps[32 * r:32 * r + 1, 512 * c:512 * (c + 1)],
                        v_sb[:, gc:gc + 1],
                        tanhT[gc][u // 2][:, 512 * (u % 2):512 * (u % 2 + 1)],
                        start=(gc == 0),
                        stop=(gc == 1),
                        tile_position=(0, 32 * r),
                    )
            state[b]["sc_ps"] = sc_ps

        def stage_softmax(b):
            sc_ps = state[b]["sc_ps"]
            # softmax (no max-shift needed: |scores| <= ||V||_1 ~ 13)
            E = ep.tile([128, 1024], f32, tag="E")
            par = smp.tile([128, 1], f32, tag="par")
            nc.scalar.activation(E[:], sc_ps[:], AF.Exp, accum_out=par[:, 0:1])
            parm = smp.tile([128, 1], f32, tag="parm")
            nc.vector.tensor_scalar(
                out=parm[:], in0=par[:], scalar1=mask[:, 0:1], scalar2=None,
                op0=ALU.mult,
            )
            sumbc = smp.tile([128, 1], f32, tag="sumbc")
            nc.gpsimd.partition_all_reduce(
                sumbc[:], parm[:], channels=128, reduce_op=bass_isa.ReduceOp.add
            )
            recip = smp.tile([128, 1], f32, tag="recip")
            nc.vector.reciprocal(recip[:], sumbc[:])

            w32 = wp.tile([128, 1024], f32, tag="w32")
            nc.vector.tensor_scalar(
                out=w32[:], in0=E[:], scalar1=recip[:, 0:1], scalar2=None,
                op0=ALU.mult,
            )
            # weights out: s = 2048*c + 512*r + f  (r = row/32, c = col/512)
            st = stp.tile([1, S], f16, tag="st")
            for c in range(2):
                nc.sync.dma_start(
                    weights_d[b:b + 1, 2048 * c:2048 * (c + 1)].rearrange(
                        "b (r f) -> b r f", r=4),
                    w32[0:97:32, 512 * c:512 * (c + 1)],
                )
                # fp16 weights row (s-ordered) for the context reduction
                nc.gpsimd.dma_start(
                    st[0:1, 2048 * c:2048 * (c + 1)].rearrange(
                        "p (r f) -> p r f", r=4),
                    w32[0:97:32, 512 * c:512 * (c + 1)],
                )
            wb = wbp.tile([128, S], f16, tag="wb")
            nc.gpsimd.partition_broadcast(wb[:], st[0:1, :])
            state[b]["wb"] = wb

        def stage_ctx(b):
            kT, wb = state[b]["kT"], state[b]["wb"]
            # context: ctx[h] = sum_s w[s] * keysT[h, s]
            for hc in range(2):
                scratch = scp.tile([128, S], f16, tag="scratch")
                nc.vector.scalar_tensor_tensor(
                    out=scratch[:],
                    in0=kT[hc][:],
                    scalar=1.0,
                    in1=wb[:],
                    op0=ALU.mult,
                    op1=ALU.mult,
                    accum_out=ctx_all[:, 2 * b + hc:2 * b + hc + 1],
                )
            del state[b]

        stage_load(0)
        stage_load(1)
        stage_kp(0)
        for b in range(BL):
            if b + 1 < BL:
                stage_kp(b + 1)
            if b + 2 < BL:
                stage_load(b + 2)
            stage_vdot(b)
            stage_softmax(b)
            stage_ctx(b)

        # context out: transpose [128, 16] -> [16, 128] and store
        ctxT_ps = sp.tile([16, 128], f32, tag="sc")
        nc.tensor.transpose(ctxT_ps[:], ctx_all[:], ident[:])
        ctxT = const.tile([16, 128], f32)
        nc.scalar.copy(ctxT[:], ctxT_ps[:])
        nc.sync.dma_start(ctx_d[:, :], ctxT[:])

    nc.compile()
    _prog_cache["nc"] = nc
    return nc


def _prep_inputs(query, keys, Wq, Wk, V):
    """Host-side marshalling: shard over batch, pre-transpose/cast."""
    query = np.asarray(query, dtype=np.float32)
    keys = np.asarray(keys, dtype=np.float32)
    Wq = np.asarray(Wq, dtype=np.float32)
    Wk = np.asarray(Wk, dtype=np.float32)
    V = np.asarray(V, dtype=np.float32)

    qp = query @ Wq.T  # [B, H] f32 (exact, tiny)
    wkT16 = np.ascontiguousarray(Wk.T).astype(np.float16)
    v16 = np.ascontiguousarray(V.reshape(2, 128).T).astype(np.float16)
    ident32 = np.eye(128, dtype=np.float32)
    keys16 = keys.astype(np.float16)

    in_maps = []
    for i in range(NCORES):
        sl = slice(BL * i, BL * (i + 1))
        keysT16 = np.ascontiguousarray(keys16[sl].transpose(0, 2, 1))  # [BL, H, S]
        # qpT_sb [128, (gc, b)]: qpT[p, gc*BL + b] = qp[b, gc*128 + p]
        qpT = np.ascontiguousarray(
            qp[sl].reshape(BL, 2, 128).transpose(2, 1, 0).reshape(128, 2 * BL)
        )
        in_maps.append({
            "keysT16": keysT16,
            "wkT16": wkT16,
            "qpT": qpT,
            "v16": v16,
            "ident32": ident32,
        })
    return in_maps


def run_device(query, keys, Wq, Wk, V, trace=False, trace_kwargs=None):
    from concourse.bass_utils import run_bass_kernel_spmd

    nc = _build_program()
    in_maps = _prep_inputs(query, keys, Wq, Wk, V)
    res = run_bass_kernel_spmd(
        nc, in_maps, list(range(NCORES)), trace=trace, **(trace_kwargs or {})
    )

    context = np.empty((B, H), dtype=np.float32)
    weights = np.empty((B, S), dtype=np.float32)
    for i in range(NCORES):
        r = res.results[i]
        sl = slice(BL * i, BL * (i + 1))
        weights[sl] = r["weights"]
        context[sl] = r["context"].reshape(BL, H)
    return (context, weights), res


def kernel(query, keys, Wq, Wk, V):
    (context, weights), _ = run_device(query, keys, Wq, Wk, V, trace=False)
    return (context, weights)


# revision 12
# speedup vs baseline: 1.0838x; 1.0314x over previous
"""Bahdanau attention on 8 Trainium2 NeuronCores (Bass/Tile), data-parallel over batch.

Problem shapes: query [64, 256] f32, keys [64, 4096, 256] f32, Wq/Wk [256, 256] f32,
V [256] f32.  Returns (context [64, 256] f32, weights [64, 4096] f32) matching

    q_proj = (query @ Wq.T)[:, None, :]
    k_proj = einsum('bsh,gh->bsg', keys, Wk)
    scores = einsum('bsh,h->bs', tanh(q_proj + k_proj), V)
    weights = softmax(scores, -1)
    context = einsum('bs,bsh->bh', weights, keys)

Sharding: batch 64 -> 8 per core; Wk/V replicated; q_proj (a [64,256] linear input
transform) is folded on the host into a per-core bias tensor.

Per-core device pipeline (B=8 local batches, S=4096, H=G=256):
  - keys are shipped pre-transposed / fp16 (keysT16 [8, 256, 4096]) so the big
    k_proj matmul can contract over h with h on partitions, and the context
    reduction can run on the Vector engine along the free (s) axis.
  - k_projT tiles [128 g, 1024 s] accumulate in PSUM over 2 h-chunks (fp16 inputs,
    fp32 accumulate); ScalarE applies tanh with the per-partition q_proj bias.
  - scores via TensorE dot with V as the 1-column stationary; each s-chunk of 512
    lands on psum row 32*(chunk%4) so one dense [128,1024] Exp (with accum_out)
    computes all 4096 exps + row sums; garbage rows are masked off and the total
    is spread to all partitions with a GpSimd partition all-reduce.
  - weights = E * (1/sum) on VectorE; DMA'd out with a strided AP that undoes the
    row-interleave.  A cast-DMA gathers the fp16 weights row, GpSimd broadcasts it
    to 128 partitions, and scalar_tensor_tensor (mult, mult, accum) reduces
    weights*keysT along s for the context.
"""

import os
import numpy as np
from contextlib import ExitStack

B, S, H = 64, 4096, 256
NCORES = 8
BL = B // NCORES  # local batches per core

_prog_cache = {}


def _build_program():
    if "nc" in _prog_cache:
        return _prog_cache["nc"]

    import concourse.bass as bass
    import concourse.tile as tile
    from concourse import bacc, mybir
    from concourse import bass_isa

    f32, f16 = mybir.dt.float32, mybir.dt.float16
    AF = mybir.ActivationFunctionType
    ALU = mybir.AluOpType

    nc = bacc.Bacc("TRN2", target_bir_lowering=False, debug=False)

    keysT_d = nc.dram_tensor("keysT16", [BL, H, S], f16, kind="ExternalInput").ap()
    wkT_d = nc.dram_tensor("wkT16", [H, H], f16, kind="ExternalInput").ap()
    qpT_d = nc.dram_tensor("qpT", [128, 2 * BL], f32, kind="ExternalInput").ap()
    v_d = nc.dram_tensor("v16", [128, 2], f16, kind="ExternalInput").ap()
    ident_d = nc.dram_tensor("ident32", [128, 128], f32, kind="ExternalInput").ap()
    weights_d = nc.dram_tensor("weights", [BL, S], f32, kind="ExternalOutput").ap()
    ctx_d = nc.dram_tensor("context", [2 * BL, 128], f32, kind="ExternalOutput").ap()

    with tile.TileContext(nc) as tc, ExitStack() as ctx:
        const = ctx.enter_context(tc.tile_pool(name="const", bufs=1))
        ktp = ctx.enter_context(tc.tile_pool(name="ktp", bufs=8))
        thp = ctx.enter_context(tc.tile_pool(name="thp", bufs=16))
        ep = ctx.enter_context(tc.tile_pool(name="ep", bufs=2))
        wp = ctx.enter_context(tc.tile_pool(name="wp", bufs=2))
        wbp = ctx.enter_context(tc.tile_pool(name="wbp", bufs=2))
        scp = ctx.enter_context(tc.tile_pool(name="scp", bufs=2))
        stp = ctx.enter_context(tc.tile_pool(name="stp", bufs=2))
        smp = ctx.enter_context(tc.tile_pool(name="smp", bufs=16))
        kpp = ctx.enter_context(tc.tile_pool(name="kpp", bufs=2, space="PSUM"))
        sp = ctx.enter_context(tc.tile_pool(name="sp", bufs=2, space="PSUM"))

        # constants / small inputs
        wkT_sb = const.tile([128, 512], f16)  # col = hc*256 + g ; h = hc*128 + p
        nc.sync.dma_start(
            wkT_sb[:].rearrange("p (hc g) -> p hc g", hc=2),
            wkT_d.rearrange("(hc p) g -> p hc g", hc=2),
        )
        qpT_sb = const.tile([128, 2 * BL], f32)  # col = gc*BL + b
        nc.sync.dma_start(qpT_sb[:], qpT_d[:, :])
        v_sb = const.tile([128, 2], f16)
        nc.sync.dma_start(v_sb[:], v_d[:, :])
        ident = const.tile([128, 128], f32)
        nc.sync.dma_start(ident[:], ident_d[:, :])
        mask = const.tile([128, 1], f32)
        nc.vector.memset(mask[:], 0.0)
        for r in range(4):
            nc.vector.memset(mask[32 * r:32 * r + 1, :], 1.0)
        negC = const.tile([128, 1], f32)
        nc.vector.memset(negC[:], -8.0)
        ctx_all = const.tile([128, 2 * BL], f32)  # col = 2*b + hc

        # --- per-batch stage emitters; emission order is software-pipelined so
        # the static per-engine instruction streams interleave batch b's tail
        # with batch b+1's matmuls (keeps TensorE dense/warm).
        state = {}

        def stage_load(b):
            kT = []
            for hc in range(2):
                t = ktp.tile([128, S], f16, tag="kT")
                nc.sync.dma_start(t[:], keysT_d[b, 128 * hc:128 * (hc + 1), :])
                kT.append(t)
            state[b] = {"kT": kT}

        def stage_kp(b):
            kT = state[b]["kT"]
            tanhT = [[None] * 4 for _ in range(2)]
            for gc in range(2):
                for v in range(4):
                    kp_ps = kpp.tile([128, 1024], f32, tag="kp")
                    for hc in range(2):
                        for du in range(2):
                            u = 2 * v + du
                            nc.tensor.matmul(
                                kp_ps[:, 512 * du:512 * (du + 1)],
                                wkT_sb[:, 256 * hc + 128 * gc:256 * hc + 128 * (gc + 1)],
                                kT[hc][:, 512 * u:512 * (u + 1)],
                                start=(hc == 0),
                                stop=(hc == 1),
                            )
                    th = thp.tile([128, 1024], f16, tag="th")
                    nc.scalar.activation(
                        th[:], kp_ps[:], AF.Tanh,
                        bias=qpT_sb[:, gc * BL + b:gc * BL + b + 1],
                    )
                    tanhT[gc][v] = th
            state[b]["tanhT"] = tanhT

        def stage_vdot(b):
            tanhT = state[b]["tanhT"]
            sc_ps = sp.tile([128, 1024], f32, tag="sc")
            if b < 2:
                # pool has 2 rotating slots; rows other than 0/32/64/96 are
                # never written afterwards, so zeroing the first use of each
                # slot keeps the masked-sum garbage rows finite forever.
                nc.vector.memset(sc_ps[:], 0.0)
            for u in range(8):
                r, c = u % 4, u // 4
                for gc in range(2):
                    nc.tensor.matmul(
                        sc_ps[32 * r:32 * r + 1, 512 * c:512 * (c + 1)],
                        v_sb[:, gc:gc + 1],
                        tanhT[gc][u // 2][:, 512 * (u % 2):512 * (u % 2 + 1)],
                        start=(gc == 0),
                        stop=(gc == 1),
                        tile_position=(0, 32 * r),
                    )
            state[b]["sc_ps"] = sc_ps

        def stage_softmax(b):
            sc_ps = state[b]["sc_ps"]
            # shifted softmax: exp(s - 8) keeps the fp16 staging copy in range;
            # the shift cancels in the normalization. (|scores| <= ||V||_1 ~ 13.)
            E = ep.tile([128, 1024], f32, tag="E")
            par = smp.tile([128, 1], f32, tag="par")
            nc.scalar.activation(E[:], sc_ps[:], AF.Exp, bias=negC[:, 0:1],
                                 accum_out=par[:, 0:1])
            # fp16 exp row (s-ordered) for the context reduction: ctx is
            # accumulated unnormalized and scaled by 1/sum afterwards, so the
            # broadcast does not wait on the sum chain.
            st = stp.tile([1, S], f16, tag="st")
            for c in range(2):
                nc.gpsimd.dma_start(
                    st[0:1, 2048 * c:2048 * (c + 1)].rearrange(
                        "p (r f) -> p r f", r=4),
                    E[0:97:32, 512 * c:512 * (c + 1)],
                )
            wb = wbp.tile([128, S], f16, tag="wb")
            nc.gpsimd.partition_broadcast(wb[:], st[0:1, :])
            state[b]["wb"] = wb

            parm = smp.tile([128, 1], f32, tag="parm")
            nc.vector.tensor_scalar(
                out=parm[:], in0=par[:], scalar1=mask[:, 0:1], scalar2=None,
                op0=ALU.mult,
            )
            sumbc = smp.tile([128, 1], f32, tag="sumbc")
            nc.gpsimd.partition_all_reduce(
                sumbc[:], parm[:], channels=128, reduce_op=bass_isa.ReduceOp.add
            )
            recip = smp.tile([128, 1], f32, tag="recip")
            nc.vector.reciprocal(recip[:], sumbc[:])
            state[b]["recip"] = recip

            w32 = wp.tile([128, 1024], f32, tag="w32")
            nc.vector.tensor_scalar(
                out=w32[:], in0=E[:], scalar1=recip[:, 0:1], scalar2=None,
                op0=ALU.mult,
            )
            # weights out: s = 2048*c + 512*r + f  (r = row/32, c = col/512)
            for c in range(2):
                nc.sync.dma_start(
                    weights_d[b:b + 1, 2048 * c:2048 * (c + 1)].rearrange(
                        "b (r f) -> b r f", r=4),
                    w32[0:97:32, 512 * c:512 * (c + 1)],
                )

        def stage_ctx(b):
            kT, wb, recip = state[b]["kT"], state[b]["wb"], state[b]["recip"]
            # context: ctx[h] = (sum_s exp16[s] * keysT[h, s]) / sum
            for hc in range(2):
                scratch = scp.tile([128, S], f16, tag="scratch")
                col = ctx_all[:, 2 * b + hc:2 * b + hc + 1]
                nc.vector.scalar_tensor_tensor(
                    out=scratch[:],
                    in0=kT[hc][:],
                    scalar=1.0,
                    in1=wb[:],
                    op0=ALU.mult,
                    op1=ALU.mult,
                    accum_out=col,
                )
                nc.vector.tensor_scalar(
                    out=col, in0=col, scalar1=recip[:, 0:1], scalar2=None,
                    op0=ALU.mult,
                )
            del state[b]

        def flush_ctx(half):
            # transpose ctx_all[:, 8h:8h+8] -> [8, 128] and store
            cols = ctx_all[:, 8 * half:8 * (half + 1)]
            ctxT_ps = sp.tile([8, 128], f32, tag="sc")
            nc.tensor.transpose(ctxT_ps[:], cols, ident[:])
            ctxT = const.tile([8, 128], f32, tag=f"ctxT{half}")
            nc.scalar.copy(ctxT[:], ctxT_ps[:])
            nc.sync.dma_start(ctx_d[8 * half:8 * (half + 1), :], ctxT[:])

        stage_load(0)
        stage_load(1)
        stage_kp(0)
        for b in range(BL):
            if b + 1 < BL:
                stage_kp(b + 1)
            if b + 2 < BL:
                stage_load(b + 2)
            stage_vdot(b)
            stage_softmax(b)
            stage_ctx(b)
            if b == BL // 2 - 1 or b == BL - 1:
                flush_ctx(0 if b == BL // 2 - 1 else 1)

    nc.compile()
    _prog_cache["nc"] = nc
    return nc


def _prep_inputs(query, keys, Wq, Wk, V):
    """Host-side marshalling: shard over batch, pre-transpose/cast."""
    query = np.asarray(query, dtype=np.float32)
    keys = np.asarray(keys, dtype=np.float32)
    Wq = np.asarray(Wq, dtype=np.float32)
    Wk = np.asarray(Wk, dtype=np.float32)
    V = np.asarray(V, dtype=np.float32)

    qp = query @ Wq.T  # [B, H] f32 (exact, tiny)
    wkT16 = np.ascontiguousarray(Wk.T).astype(np.float16)
    v16 = np.ascontiguousarray(V.reshape(2, 128).T).astype(np.float16)
    ident32 = np.eye(128, dtype=np.float32)
    keys16 = keys.astype(np.float16)

    in_maps = []
    for i in range(NCORES):
        sl = slice(BL * i, BL * (i + 1))
        keysT16 = np.ascontiguousarray(keys16[sl].transpose(0, 2, 1))  # [BL, H, S]
        # qpT_sb [128, (gc, b)]: qpT[p, gc*BL + b] = qp[b, gc*128 + p]
        qpT = np.ascontiguousarray(
            qp[sl].reshape(BL, 2, 128).transpose(2, 1, 0).reshape(128, 2 * BL)
        )
        in_maps.append({
            "keysT16": keysT16,
            "wkT16": wkT16,
            "qpT": qpT,
            "v16": v16,
            "ident32": ident32,
        })
    return in_maps


def run_device(query, keys, Wq, Wk, V, trace=False, trace_kwargs=None):
    from concourse.bass_utils import run_bass_kernel_spmd

    nc = _build_program()
    in_maps = _prep_inputs(query, keys, Wq, Wk, V)
    res = run_bass_kernel_spmd(
        nc, in_maps, list(range(NCORES)), trace=trace, **(trace_kwargs or {})
    )

    context = np.empty((B, H), dtype=np.float32)
    weights = np.empty((B, S), dtype=np.float32)
    for i in range(NCORES):
        r = res.results[i]
        sl = slice(BL * i, BL * (i + 1))
        weights[sl] = r["weights"]
        context[sl] = r["context"].reshape(BL, H)
    return (context, weights), res


def kernel(query, keys, Wq, Wk, V):
    (context, weights), _ = run_device(query, keys, Wq, Wk, V, trace=False)
    return (context, weights)


# revision 14
# speedup vs baseline: 1.0878x; 1.0036x over previous
"""Bahdanau attention on 8 Trainium2 NeuronCores (Bass/Tile), data-parallel over batch.

Problem shapes: query [64, 256] f32, keys [64, 4096, 256] f32, Wq/Wk [256, 256] f32,
V [256] f32.  Returns (context [64, 256] f32, weights [64, 4096] f32) matching

    q_proj = (query @ Wq.T)[:, None, :]
    k_proj = einsum('bsh,gh->bsg', keys, Wk)
    scores = einsum('bsh,h->bs', tanh(q_proj + k_proj), V)
    weights = softmax(scores, -1)
    context = einsum('bs,bsh->bh', weights, keys)

Sharding: batch 64 -> 8 per core; Wk/V replicated; q_proj (a [64,256] linear input
transform) is folded on the host into a per-core bias tensor.

Per-core device pipeline (B=8 local batches, S=4096, H=G=256):
  - keys are shipped pre-transposed / fp16 (keysT16 [8, 256, 4096]) so the big
    k_proj matmul can contract over h with h on partitions, and the context
    reduction can run on the Vector engine along the free (s) axis.
  - k_projT tiles [128 g, 1024 s] accumulate in PSUM over 2 h-chunks (fp16 inputs,
    fp32 accumulate); ScalarE applies tanh with the per-partition q_proj bias.
  - scores via TensorE dot with V as the 1-column stationary; each s-chunk of 512
    lands on psum row 32*(chunk%4) so one dense [128,1024] Exp (with accum_out)
    computes all 4096 exps + row sums; garbage rows are masked off and the total
    is spread to all partitions with a GpSimd partition all-reduce.
  - weights = E * (1/sum) on VectorE; DMA'd out with a strided AP that undoes the
    row-interleave.  A cast-DMA gathers the fp16 weights row, GpSimd broadcasts it
    to 128 partitions, and scalar_tensor_tensor (mult, mult, accum) reduces
    weights*keysT along s for the context.
"""

import os
import numpy as np
from contextlib import ExitStack

B, S, H = 64, 4096, 256
NCORES = 8
BL = B // NCORES  # local batches per core

_prog_cache = {}


def _build_program():
    if "nc" in _prog_cache:
        return _prog_cache["nc"]

    import concourse.bass as bass
    import concourse.tile as tile
    from concourse import bacc, mybir
    from concourse import bass_isa

    f32, f16 = mybir.dt.float32, mybir.dt.float16
    AF = mybir.ActivationFunctionType
    ALU = mybir.AluOpType

    nc = bacc.Bacc("TRN2", target_bir_lowering=False, debug=False)

    keysT_d = nc.dram_tensor("keysT16", [BL, H, S], f16, kind="ExternalInput").ap()
    wkT_d = nc.dram_tensor("wkT16", [H, H], f16, kind="ExternalInput").ap()
    qpT_d = nc.dram_tensor("qpT", [128, 2 * BL], f32, kind="ExternalInput").ap()
    v_d = nc.dram_tensor("v16", [128, 2], f16, kind="ExternalInput").ap()
    ident_d = nc.dram_tensor("ident32", [128, 128], f32, kind="ExternalInput").ap()
    weights_d = nc.dram_tensor("weights", [BL, S], f32, kind="ExternalOutput").ap()
    ctx_d = nc.dram_tensor("context", [2 * BL, 128], f32, kind="ExternalOutput").ap()

    with tile.TileContext(nc) as tc, ExitStack() as ctx:
        const = ctx.enter_context(tc.tile_pool(name="const", bufs=1))
        ktp = ctx.enter_context(tc.tile_pool(name="ktp", bufs=8))
        thp = ctx.enter_context(tc.tile_pool(name="thp", bufs=24))
        ep = ctx.enter_context(tc.tile_pool(name="ep", bufs=2))
        wp = ctx.enter_context(tc.tile_pool(name="wp", bufs=2))
        wbp = ctx.enter_context(tc.tile_pool(name="wbp", bufs=2))
        scp = ctx.enter_context(tc.tile_pool(name="scp", bufs=2))
        stp = ctx.enter_context(tc.tile_pool(name="stp", bufs=2))
        smp = ctx.enter_context(tc.tile_pool(name="smp", bufs=16))
        kpp = ctx.enter_context(tc.tile_pool(name="kpp", bufs=2, space="PSUM"))
        sp = ctx.enter_context(tc.tile_pool(name="sp", bufs=2, space="PSUM"))

        # constants / small inputs
        wkT_sb = const.tile([128, 512], f16)  # col = hc*256 + g ; h = hc*128 + p
        nc.sync.dma_start(
            wkT_sb[:].rearrange("p (hc g) -> p hc g", hc=2),
            wkT_d.rearrange("(hc p) g -> p hc g", hc=2),
        )
        qpT_sb = const.tile([128, 2 * BL], f32)  # col = gc*BL + b
        nc.sync.dma_start(qpT_sb[:], qpT_d[:, :])
        v_sb = const.tile([128, 2], f16)
        nc.sync.dma_start(v_sb[:], v_d[:, :])
        ident = const.tile([128, 128], f32)
        nc.sync.dma_start(ident[:], ident_d[:, :])
        mask = const.tile([128, 1], f32)
        nc.vector.memset(mask[:], 0.0)
        for r in range(4):
            nc.vector.memset(mask[32 * r:32 * r + 1, :], 1.0)
        negC = const.tile([128, 1], f32)
        nc.vector.memset(negC[:], -8.0)
        ctx_all = const.tile([128, 2 * BL], f32)  # col = 2*b + hc

        # --- per-batch stage emitters; emission order is software-pipelined so
        # the static per-engine instruction streams interleave batch b's tail
        # with batch b+1's matmuls (keeps TensorE dense/warm).
        state = {}

        def stage_load(b):
            kT = []
            for hc in range(2):
                t = ktp.tile([128, S], f16, tag="kT")
                nc.sync.dma_start(t[:], keysT_d[b, 128 * hc:128 * (hc + 1), :])
                kT.append(t)
            state[b] = {"kT": kT}

        def stage_kp(b):
            kT = state[b]["kT"]
            tanhT = [[None] * 4 for _ in range(2)]
            for gc in range(2):
                for v in range(4):
                    kp_ps = kpp.tile([128, 1024], f32, tag="kp")
                    for hc in range(2):
                        for du in range(2):
                            u = 2 * v + du
                            nc.tensor.matmul(
                                kp_ps[:, 512 * du:512 * (du + 1)],
                                wkT_sb[:, 256 * hc + 128 * gc:256 * hc + 128 * (gc + 1)],
                                kT[hc][:, 512 * u:512 * (u + 1)],
                                start=(hc == 0),
                                stop=(hc == 1),
                            )
                    th = thp.tile([128, 1024], f16, tag="th")
                    nc.scalar.activation(
                        th[:], kp_ps[:], AF.Tanh,
                        bias=qpT_sb[:, gc * BL + b:gc * BL + b + 1],
                    )
                    tanhT[gc][v] = th
            state[b]["tanhT"] = tanhT

        def stage_vdot(b):
            tanhT = state[b]["tanhT"]
            sc_ps = sp.tile([128, 1024], f32, tag="sc")
            if b < 2:
                # pool has 2 rotating slots; rows other than 0/32/64/96 are
                # never written afterwards, so zeroing the first use of each
                # slot keeps the masked-sum garbage rows finite forever.
                nc.vector.memset(sc_ps[:], 0.0)
            for u in range(8):
                r, c = u % 4, u // 4
                for gc in range(2):
                    nc.tensor.matmul(
                        sc_ps[32 * r:32 * r + 1, 512 * c:512 * (c + 1)],
                        v_sb[:, gc:gc + 1],
                        tanhT[gc][u // 2][:, 512 * (u % 2):512 * (u % 2 + 1)],
                        start=(gc == 0),
                        stop=(gc == 1),
                        tile_position=(0, 32 * r),
                    )
            state[b]["sc_ps"] = sc_ps

        def stage_softmax(b):
            sc_ps = state[b]["sc_ps"]
            # shifted softmax: exp(s - 8) keeps the fp16 staging copy in range;
            # the shift cancels in the normalization. (|scores| <= ||V||_1 ~ 13.)
            E = ep.tile([128, 1024], f32, tag="E")
            par = smp.tile([128, 1], f32, tag="par")
            nc.scalar.activation(E[:], sc_ps[:], AF.Exp, bias=negC[:, 0:1],
                                 accum_out=par[:, 0:1])
            # fp16 exp row (s-ordered) for the context reduction: ctx is
            # accumulated unnormalized and scaled by 1/sum afterwards, so the
            # broadcast does not wait on the sum chain.
            st = stp.tile([1, S], f16, tag="st")
            for c in range(2):
                nc.gpsimd.dma_start(
                    st[0:1, 2048 * c:2048 * (c + 1)].rearrange(
                        "p (r f) -> p r f", r=4),
                    E[0:97:32, 512 * c:512 * (c + 1)],
                )
            wb = wbp.tile([128, S], f16, tag="wb")
            nc.gpsimd.partition_broadcast(wb[:], st[0:1, :])
            state[b]["wb"] = wb

            parm = smp.tile([128, 1], f32, tag="parm")
            nc.vector.tensor_scalar(
                out=parm[:], in0=par[:], scalar1=mask[:, 0:1], scalar2=None,
                op0=ALU.mult,
            )
            sumbc = smp.tile([128, 1], f32, tag="sumbc")
            nc.gpsimd.partition_all_reduce(
                sumbc[:], parm[:], channels=128, reduce_op=bass_isa.ReduceOp.add
            )
            recip = smp.tile([128, 1], f32, tag="recip")
            nc.vector.reciprocal(recip[:], sumbc[:])
            state[b]["recip"] = recip

            w32 = wp.tile([128, 1024], f32, tag="w32")
            nc.vector.tensor_scalar(
                out=w32[:], in0=E[:], scalar1=recip[:, 0:1], scalar2=None,
                op0=ALU.mult,
            )
            # weights out: s = 2048*c + 512*r + f  (r = row/32, c = col/512)
            for c in range(2):
                nc.sync.dma_start(
                    weights_d[b:b + 1, 2048 * c:2048 * (c + 1)].rearrange(
                        "b (r f) -> b r f", r=4),
                    w32[0:97:32, 512 * c:512 * (c + 1)],
                )

        def stage_ctx(b):
            kT, wb, recip = state[b]["kT"], state[b]["wb"], state[b]["recip"]
            # context: ctx[h] = (sum_s exp16[s] * keysT[h, s]) / sum
            for hc in range(2):
                scratch = scp.tile([128, S], f16, tag="scratch")
                col = ctx_all[:, 2 * b + hc:2 * b + hc + 1]
                nc.vector.scalar_tensor_tensor(
                    out=scratch[:],
                    in0=kT[hc][:],
                    scalar=1.0,
                    in1=wb[:],
                    op0=ALU.mult,
                    op1=ALU.mult,
                    accum_out=col,
                )
                nc.vector.tensor_scalar(
                    out=col, in0=col, scalar1=recip[:, 0:1], scalar2=None,
                    op0=ALU.mult,
                )
            del state[b]

        def flush_ctx(half):
            # transpose ctx_all[:, 8h:8h+8] -> [8, 128] and store
            cols = ctx_all[:, 8 * half:8 * (half + 1)]
            ctxT_ps = sp.tile([8, 128], f32, tag="sc")
            nc.tensor.transpose(ctxT_ps[:], cols, ident[:])
            ctxT = const.tile([8, 128], f32, tag=f"ctxT{half}")
            nc.scalar.copy(ctxT[:], ctxT_ps[:])
            nc.sync.dma_start(ctx_d[8 * half:8 * (half + 1), :], ctxT[:])

        stage_load(0)
        stage_load(1)
        stage_kp(0)
        stage_kp(1)
        stage_load(2)
        for b in range(BL):
            if b + 2 < BL:
                stage_kp(b + 2)
            if b + 3 < BL:
                stage_load(b + 3)
            stage_vdot(b)
            stage_softmax(b)
            stage_ctx(b)
            if b == BL - 3:
                flush_ctx(0)
        flush_ctx(1)

    nc.compile()
    _prog_cache["nc"] = nc
    return nc


def _prep_inputs(query, keys, Wq, Wk, V):
    """Host-side marshalling: shard over batch, pre-transpose/cast."""
    query = np.asarray(query, dtype=np.float32)
    keys = np.asarray(keys, dtype=np.float32)
    Wq = np.asarray(Wq, dtype=np.float32)
    Wk = np.asarray(Wk, dtype=np.float32)
    V = np.asarray(V, dtype=np.float32)

    qp = query @ Wq.T  # [B, H] f32 (exact, tiny)
    wkT16 = np.ascontiguousarray(Wk.T).astype(np.float16)
    v16 = np.ascontiguousarray(V.reshape(2, 128).T).astype(np.float16)
    ident32 = np.eye(128, dtype=np.float32)
    keys16 = keys.astype(np.float16)

    in_maps = []
    for i in range(NCORES):
        sl = slice(BL * i, BL * (i + 1))
        keysT16 = np.ascontiguousarray(keys16[sl].transpose(0, 2, 1))  # [BL, H, S]
        # qpT_sb [128, (gc, b)]: qpT[p, gc*BL + b] = qp[b, gc*128 + p]
        qpT = np.ascontiguousarray(
            qp[sl].reshape(BL, 2, 128).transpose(2, 1, 0).reshape(128, 2 * BL)
        )
        in_maps.append({
            "keysT16": keysT16,
            "wkT16": wkT16,
            "qpT": qpT,
            "v16": v16,
            "ident32": ident32,
        })
    return in_maps


def run_device(query, keys, Wq, Wk, V, trace=False, trace_kwargs=None):
    from concourse.bass_utils import run_bass_kernel_spmd

    nc = _build_program()
    in_maps = _prep_inputs(query, keys, Wq, Wk, V)
    res = run_bass_kernel_spmd(
        nc, in_maps, list(range(NCORES)), trace=trace, **(trace_kwargs or {})
    )

    context = np.empty((B, H), dtype=np.float32)
    weights = np.empty((B, S), dtype=np.float32)
    for i in range(NCORES):
        r = res.results[i]
        sl = slice(BL * i, BL * (i + 1))
        weights[sl] = r["weights"]
        context[sl] = r["context"].reshape(BL, H)
    return (context, weights), res


def kernel(query, keys, Wq, Wk, V):
    (context, weights), _ = run_device(query, keys, Wq, Wk, V, trace=False)
    return (context, weights)


# revision 16
# speedup vs baseline: 1.1257x; 1.0348x over previous
"""Bahdanau attention on 8 Trainium2 NeuronCores (Bass/Tile), data-parallel over batch.

Problem shapes: query [64, 256] f32, keys [64, 4096, 256] f32, Wq/Wk [256, 256] f32,
V [256] f32.  Returns (context [64, 256] f32, weights [64, 4096] f32) matching

    q_proj = (query @ Wq.T)[:, None, :]
    k_proj = einsum('bsh,gh->bsg', keys, Wk)
    scores = einsum('bsh,h->bs', tanh(q_proj + k_proj), V)
    weights = softmax(scores, -1)
    context = einsum('bs,bsh->bh', weights, keys)

Sharding: batch 64 -> 8 per core; Wk/V replicated; q_proj (a [64,256] linear input
transform) is folded on the host into a per-core bias tensor.

Per-core device pipeline (B=8 local batches, S=4096, H=G=256):
  - keys are shipped pre-transposed / fp16 (keysT16 [8, 256, 4096]) so the big
    k_proj matmul can contract over h with h on partitions, and the context
    reduction can run on the Vector engine along the free (s) axis.
  - k_projT tiles [128 g, 1024 s] accumulate in PSUM over 2 h-chunks (fp16 inputs,
    fp32 accumulate); ScalarE applies tanh with the per-partition q_proj bias.
  - scores via TensorE dot with V as the 1-column stationary; each s-chunk of 512
    lands on psum row 32*(chunk%4) so one dense [128,1024] Exp (with accum_out)
    computes all 4096 exps + row sums; garbage rows are masked off and the total
    is spread to all partitions with a GpSimd partition all-reduce.
  - weights = E * (1/sum) on VectorE; DMA'd out with a strided AP that undoes the
    row-interleave.  A cast-DMA gathers the fp16 weights row, GpSimd broadcasts it
    to 128 partitions, and scalar_tensor_tensor (mult, mult, accum) reduces
    weights*keysT along s for the context.
"""

import os
import numpy as np
from contextlib import ExitStack

B, S, H = 64, 4096, 256
NCORES = 8
BL = B // NCORES  # local batches per core

_prog_cache = {}


def _build_program():
    if "nc" in _prog_cache:
        return _prog_cache["nc"]

    import concourse.bass as bass
    import concourse.tile as tile
    from concourse import bacc, mybir
    from concourse import bass_isa

    f32, f16 = mybir.dt.float32, mybir.dt.float16
    AF = mybir.ActivationFunctionType
    ALU = mybir.AluOpType

    nc = bacc.Bacc("TRN2", target_bir_lowering=False, debug=False)

    keysT_d = nc.dram_tensor("keysT16", [BL, H, S], f16, kind="ExternalInput").ap()
    wkT_d = nc.dram_tensor("wkT16", [H, H], f16, kind="ExternalInput").ap()
    qpT_d = nc.dram_tensor("qpT", [128, 2 * BL], f32, kind="ExternalInput").ap()
    v_d = nc.dram_tensor("v16", [128, 2], f16, kind="ExternalInput").ap()
    ident_d = nc.dram_tensor("ident32", [128, 128], f32, kind="ExternalInput").ap()
    weights_d = nc.dram_tensor("weights", [BL, S], f32, kind="ExternalOutput").ap()
    ctx_d = nc.dram_tensor("context", [2 * BL, 128], f32, kind="ExternalOutput").ap()

    with tile.TileContext(nc) as tc, ExitStack() as ctx:
        const = ctx.enter_context(tc.tile_pool(name="const", bufs=1))
        ktp = ctx.enter_context(tc.tile_pool(name="ktp", bufs=8))
        thp = ctx.enter_context(tc.tile_pool(name="thp", bufs=24))
        ep = ctx.enter_context(tc.tile_pool(name="ep", bufs=2))
        wp = ctx.enter_context(tc.tile_pool(name="wp", bufs=2))
        wbp = ctx.enter_context(tc.tile_pool(name="wbp", bufs=2))
        scp = ctx.enter_context(tc.tile_pool(name="scp", bufs=2))
        stp = ctx.enter_context(tc.tile_pool(name="stp", bufs=2))
        smp = ctx.enter_context(tc.tile_pool(name="smp", bufs=16))
        kpp = ctx.enter_context(tc.tile_pool(name="kpp", bufs=2, space="PSUM"))
        sp = ctx.enter_context(tc.tile_pool(name="sp", bufs=2, space="PSUM"))

        # constants / small inputs
        wkT_sb = const.tile([128, 512], f16)  # col = hc*256 + g ; h = hc*128 + p
        nc.sync.dma_start(
            wkT_sb[:].rearrange("p (hc g) -> p hc g", hc=2),
            wkT_d.rearrange("(hc p) g -> p hc g", hc=2),
        )
        qpT_sb = const.tile([128, 2 * BL], f32)  # col = gc*BL + b
        nc.sync.dma_start(qpT_sb[:], qpT_d[:, :])
        v_sb = const.tile([128, 2], f16)
        nc.sync.dma_start(v_sb[:], v_d[:, :])
        ident = const.tile([128, 128], f32)
        nc.sync.dma_start(ident[:], ident_d[:, :])
        mask = const.tile([128, 1], f32)
        nc.vector.memset(mask[:], 0.0)
        for r in range(4):
            nc.vector.memset(mask[32 * r:32 * r + 1, :], 1.0)
        negC = const.tile([128, 1], f32)
        nc.vector.memset(negC[:], -8.0)
        ctx_all = const.tile([128, 2 * BL], f32)  # col = 2*b + hc

        # --- per-batch stage emitters; emission order is software-pipelined so
        # the static per-engine instruction streams interleave batch b's tail
        # with batch b+1's matmuls (keeps TensorE dense/warm).
        state = {}

        def stage_load(b):
            kT = []
            for hc in range(2):
                t = ktp.tile([128, S], f16, tag="kT")
                nc.sync.dma_start(t[:], keysT_d[b, 128 * hc:128 * (hc + 1), :])
                kT.append(t)
            state[b] = {"kT": kT}

        def stage_kp(b):
            kT = state[b]["kT"]
            tanhT = [[None] * 4 for _ in range(2)]
            for gc in range(2):
                for v in range(4):
                    kp_ps = kpp.tile([128, 1024], f32, tag="kp")
                    for hc in range(2):
                        for du in range(2):
                            u = 2 * v + du
                            nc.tensor.matmul(
                                kp_ps[:, 512 * du:512 * (du + 1)],
                                wkT_sb[:, 256 * hc + 128 * gc:256 * hc + 128 * (gc + 1)],
                                kT[hc][:, 512 * u:512 * (u + 1)],
                                start=(hc == 0),
                                stop=(hc == 1),
                            )
                    th = thp.tile([128, 1024], f16, tag="th")
                    nc.scalar.activation(
                        th[:], kp_ps[:], AF.Tanh,
                        bias=qpT_sb[:, gc * BL + b:gc * BL + b + 1],
                    )
                    tanhT[gc][v] = th
            state[b]["tanhT"] = tanhT

        def stage_vdot(b):
            tanhT = state[b]["tanhT"]
            sc_ps = sp.tile([128, 1024], f32, tag="sc")
            if b < 2:
                # pool has 2 rotating slots; rows other than 0/32/64/96 are
                # never written afterwards, so zeroing the first use of each
                # slot keeps the masked-sum garbage rows finite forever.
                nc.vector.memset(sc_ps[:], 0.0)
            # gc-outer so consecutive matmuls hit different column groups
            # (4-way tile_position packing) while keeping start before stop
            # within each psum region.
            for gc in range(2):
                for u in range(8):
                    r, c = u % 4, u // 4
                    nc.tensor.matmul(
                        sc_ps[32 * r:32 * r + 1, 512 * c:512 * (c + 1)],
                        v_sb[:, gc:gc + 1],
                        tanhT[gc][u // 2][:, 512 * (u % 2):512 * (u % 2 + 1)],
                        start=(gc == 0),
                        stop=(gc == 1),
                        tile_position=(0, 32 * r),
                    )
            state[b]["sc_ps"] = sc_ps

        def stage_softmax(b):
            sc_ps = state[b]["sc_ps"]
            # shifted softmax: exp(s - 8) keeps the fp16 staging copy in range;
            # the shift cancels in the normalization. (|scores| <= ||V||_1 ~ 13.)
            E = ep.tile([128, 1024], f32, tag="E")
            par = smp.tile([128, 1], f32, tag="par")
            nc.scalar.activation(E[:], sc_ps[:], AF.Exp, bias=negC[:, 0:1],
                                 accum_out=par[:, 0:1])
            # fp16 exp row (s-ordered) for the context reduction: ctx is
            # accumulated unnormalized and scaled by 1/sum afterwards, so the
            # broadcast does not wait on the sum chain.
            st = stp.tile([1, S], f16, tag="st")
            for c in range(2):
                nc.gpsimd.dma_start(
                    st[0:1, 2048 * c:2048 * (c + 1)].rearrange(
                        "p (r f) -> p r f", r=4),
                    E[0:97:32, 512 * c:512 * (c + 1)],
                )
            wb = wbp.tile([128, S], f16, tag="wb")
            nc.gpsimd.partition_broadcast(wb[:], st[0:1, :])
            state[b]["wb"] = wb

            parm = smp.tile([128, 1], f32, tag="parm")
            nc.vector.tensor_scalar(
                out=parm[:], in0=par[:], scalar1=mask[:, 0:1], scalar2=None,
                op0=ALU.mult,
            )
            sumbc = smp.tile([128, 1], f32, tag="sumbc")
            nc.gpsimd.partition_all_reduce(
                sumbc[:], parm[:], channels=128, reduce_op=bass_isa.ReduceOp.add
            )
            recip = smp.tile([128, 1], f32, tag="recip")
            nc.vector.reciprocal(recip[:], sumbc[:])
            state[b]["recip"] = recip

            w32 = wp.tile([128, 1024], f32, tag="w32")
            nc.vector.tensor_scalar(
                out=w32[:], in0=E[:], scalar1=recip[:, 0:1], scalar2=None,
                op0=ALU.mult,
            )
            # weights out: s = 2048*c + 512*r + f  (r = row/32, c = col/512)
            for c in range(2):
                nc.sync.dma_start(
                    weights_d[b:b + 1, 2048 * c:2048 * (c + 1)].rearrange(
                        "b (r f) -> b r f", r=4),
                    w32[0:97:32, 512 * c:512 * (c + 1)],
                )

        def stage_ctx(b):
            kT, wb, recip = state[b]["kT"], state[b]["wb"], state[b]["recip"]
            # context: ctx[h] = (sum_s exp16[s] * keysT[h, s]) / sum
            for hc in range(2):
                scratch = scp.tile([128, S], f16, tag="scratch")
                col = ctx_all[:, 2 * b + hc:2 * b + hc + 1]
                nc.vector.scalar_tensor_tensor(
                    out=scratch[:],
                    in0=kT[hc][:],
                    scalar=1.0,
                    in1=wb[:],
                    op0=ALU.mult,
                    op1=ALU.mult,
                    accum_out=col,
                )
                nc.vector.tensor_scalar(
                    out=col, in0=col, scalar1=recip[:, 0:1], scalar2=None,
                    op0=ALU.mult,
                )
            del state[b]

        def flush_ctx(half):
            # transpose ctx_all[:, 8h:8h+8] -> [8, 128] and store
            cols = ctx_all[:, 8 * half:8 * (half + 1)]
            ctxT_ps = sp.tile([8, 128], f32, tag="sc")
            nc.tensor.transpose(ctxT_ps[:], cols, ident[:])
            ctxT = const.tile([8, 128], f32, tag=f"ctxT{half}")
            nc.scalar.copy(ctxT[:], ctxT_ps[:])
            nc.sync.dma_start(ctx_d[8 * half:8 * (half + 1), :], ctxT[:])

        stage_load(0)
        stage_load(1)
        stage_kp(0)
        for b in range(BL):
            if b + 1 < BL:
                stage_kp(b + 1)
            if b + 2 < BL:
                stage_load(b + 2)
            stage_vdot(b)
            stage_softmax(b)
            stage_ctx(b)
            if b == BL - 3:
                flush_ctx(0)
        flush_ctx(1)

    nc.compile()
    _prog_cache["nc"] = nc
    return nc


def _prep_inputs(query, keys, Wq, Wk, V):
    """Host-side marshalling: shard over batch, pre-transpose/cast."""
    query = np.asarray(query, dtype=np.float32)
    keys = np.asarray(keys, dtype=np.float32)
    Wq = np.asarray(Wq, dtype=np.float32)
    Wk = np.asarray(Wk, dtype=np.float32)
    V = np.asarray(V, dtype=np.float32)

    qp = query @ Wq.T  # [B, H] f32 (exact, tiny)
    wkT16 = np.ascontiguousarray(Wk.T).astype(np.float16)
    v16 = np.ascontiguousarray(V.reshape(2, 128).T).astype(np.float16)
    ident32 = np.eye(128, dtype=np.float32)
    keys16 = keys.astype(np.float16)

    in_maps = []
    for i in range(NCORES):
        sl = slice(BL * i, BL * (i + 1))
        keysT16 = np.ascontiguousarray(keys16[sl].transpose(0, 2, 1))  # [BL, H, S]
        # qpT_sb [128, (gc, b)]: qpT[p, gc*BL + b] = qp[b, gc*128 + p]
        qpT = np.ascontiguousarray(
            qp[sl].reshape(BL, 2, 128).transpose(2, 1, 0).reshape(128, 2 * BL)
        )
        in_maps.append({
            "keysT16": keysT16,
            "wkT16": wkT16,
            "qpT": qpT,
            "v16": v16,
            "ident32": ident32,
        })
    return in_maps


def run_device(query, keys, Wq, Wk, V, trace=False, trace_kwargs=None):
    from concourse.bass_utils import run_bass_kernel_spmd

    nc = _build_program()
    in_maps = _prep_inputs(query, keys, Wq, Wk, V)
    res = run_bass_kernel_spmd(
        nc, in_maps, list(range(NCORES)), trace=trace, **(trace_kwargs or {})
    )

    context = np.empty((B, H), dtype=np.float32)
    weights = np.empty((B, S), dtype=np.float32)
    for i in range(NCORES):
        r = res.results[i]
        sl = slice(BL * i, BL * (i + 1))
        weights[sl] = r["weights"]
        context[sl] = r["context"].reshape(BL, H)
    return (context, weights), res


def kernel(query, keys, Wq, Wk, V):
    (context, weights), _ = run_device(query, keys, Wq, Wk, V, trace=False)
    return (context, weights)


# revision 23
# speedup vs baseline: 1.1334x; 1.0069x over previous
"""Bahdanau attention on 8 Trainium2 NeuronCores (Bass/Tile), data-parallel over batch.

Problem shapes: query [64, 256] f32, keys [64, 4096, 256] f32, Wq/Wk [256, 256] f32,
V [256] f32.  Returns (context [64, 256] f32, weights [64, 4096] f32) matching

    q_proj = (query @ Wq.T)[:, None, :]
    k_proj = einsum('bsh,gh->bsg', keys, Wk)
    scores = einsum('bsh,h->bs', tanh(q_proj + k_proj), V)
    weights = softmax(scores, -1)
    context = einsum('bs,bsh->bh', weights, keys)

Sharding: batch 64 -> 8 per core; Wk/V replicated; q_proj (a [64,256] linear input
transform) is folded on the host into a per-core bias tensor.

Per-core device pipeline (B=8 local batches, S=4096, H=G=256):
  - keys are shipped pre-transposed / fp16 (keysT16 [8, 256, 4096]) so the big
    k_proj matmul can contract over h with h on partitions, and the context
    reduction can run on the Vector engine along the free (s) axis.
  - k_projT tiles [128 g, 1024 s] accumulate in PSUM over 2 h-chunks (fp16 inputs,
    fp32 accumulate); ScalarE applies tanh with the per-partition q_proj bias.
  - scores via TensorE dot with V as the 1-column stationary; each s-chunk of 512
    lands on psum row 32*(chunk%4) so one dense [128,1024] Exp (with accum_out)
    computes all 4096 exps + row sums; garbage rows are masked off and the total
    is spread to all partitions with a GpSimd partition all-reduce.
  - weights = E * (1/sum) on VectorE; DMA'd out with a strided AP that undoes the
    row-interleave.  A cast-DMA gathers the fp16 weights row, GpSimd broadcasts it
    to 128 partitions, and scalar_tensor_tensor (mult, mult, accum) reduces
    weights*keysT along s for the context.
"""

import os
import numpy as np
from contextlib import ExitStack

B, S, H = 64, 4096, 256
NCORES = 8
BL = B // NCORES  # local batches per core

_prog_cache = {}


def _build_program():
    if "nc" in _prog_cache:
        return _prog_cache["nc"]

    import concourse.bass as bass
    import concourse.tile as tile
    from concourse import bacc, mybir
    from concourse import bass_isa

    f32, f16 = mybir.dt.float32, mybir.dt.float16
    AF = mybir.ActivationFunctionType
    ALU = mybir.AluOpType

    nc = bacc.Bacc("TRN2", target_bir_lowering=False, debug=False)

    keysT_d = nc.dram_tensor("keysT16", [BL, H, S], f16, kind="ExternalInput").ap()
    wkT_d = nc.dram_tensor("wkT16", [H, H], f16, kind="ExternalInput").ap()
    qpT_d = nc.dram_tensor("qpT", [128, 2 * BL], f32, kind="ExternalInput").ap()
    v_d = nc.dram_tensor("v16", [128, 2], f16, kind="ExternalInput").ap()
    ident_d = nc.dram_tensor("ident32", [128, 128], f32, kind="ExternalInput").ap()
    weights_d = nc.dram_tensor("weights", [BL, S], f32, kind="ExternalOutput").ap()
    ctx_d = nc.dram_tensor("context", [2 * BL, 128], f32, kind="ExternalOutput").ap()

    with tile.TileContext(nc) as tc, ExitStack() as ctx:
        const = ctx.enter_context(tc.tile_pool(name="const", bufs=1))
        ktp = ctx.enter_context(tc.tile_pool(name="ktp", bufs=8))
        thp = ctx.enter_context(tc.tile_pool(name="thp", bufs=24))
        ep = ctx.enter_context(tc.tile_pool(name="ep", bufs=2))
        wp = ctx.enter_context(tc.tile_pool(name="wp", bufs=2))
        wbp = ctx.enter_context(tc.tile_pool(name="wbp", bufs=2))
        scp = ctx.enter_context(tc.tile_pool(name="scp", bufs=2))
        stp = ctx.enter_context(tc.tile_pool(name="stp", bufs=2))
        smp = ctx.enter_context(tc.tile_pool(name="smp", bufs=16))
        kpp = ctx.enter_context(tc.tile_pool(name="kpp", bufs=2, space="PSUM"))
        sp = ctx.enter_context(tc.tile_pool(name="sp", bufs=2, space="PSUM"))

        # constants / small inputs
        wkT_sb = const.tile([128, 512], f16)  # col = hc*256 + g ; h = hc*128 + p
        nc.sync.dma_start(
            wkT_sb[:].rearrange("p (hc g) -> p hc g", hc=2),
            wkT_d.rearrange("(hc p) g -> p hc g", hc=2),
        )
        qpT_sb = const.tile([128, 2 * BL], f32)  # col = gc*BL + b
        nc.sync.dma_start(qpT_sb[:], qpT_d[:, :])
        v_sb = const.tile([128, 2], f16)
        nc.sync.dma_start(v_sb[:], v_d[:, :])
        ident = const.tile([128, 128], f32)
        nc.sync.dma_start(ident[:], ident_d[:, :])
        mask = const.tile([128, 1], f32)
        nc.vector.memset(mask[:], 0.0)
        for r in range(4):
            nc.vector.memset(mask[32 * r:32 * r + 1, :], 1.0)
        negC = const.tile([128, 1], f32)
        nc.vector.memset(negC[:], -8.0)
        recipT0 = const.tile([8, 1], f32, tag="recipT0")
        recipT1 = const.tile([8, 1], f32, tag="recipT1")
        recipT = [recipT0, recipT1]
        ctx_all = const.tile([128, 2 * BL], f32)  # col = 2*b + hc

        # --- per-batch stage emitters; emission order is software-pipelined so
        # the static per-engine instruction streams interleave batch b's tail
        # with batch b+1's matmuls (keeps TensorE dense/warm).
        state = {}

        def stage_load(b):
            kT = []
            for hc in range(2):
                t = ktp.tile([128, S], f16, tag="kT")
                nc.sync.dma_start(t[:], keysT_d[b, 128 * hc:128 * (hc + 1), :])
                kT.append(t)
            state[b] = {"kT": kT}

        def stage_kp(b):
            kT = state[b]["kT"]
            tanhT = [[None] * 4 for _ in range(2)]
            for gc in range(2):
                for v in range(4):
                    kp_ps = kpp.tile([128, 1024], f32, tag="kp")
                    for hc in range(2):
                        for du in range(2):
                            u = 2 * v + du
                            nc.tensor.matmul(
                                kp_ps[:, 512 * du:512 * (du + 1)],
                                wkT_sb[:, 256 * hc + 128 * gc:256 * hc + 128 * (gc + 1)],
                                kT[hc][:, 512 * u:512 * (u + 1)],
                                start=(hc == 0),
                                stop=(hc == 1),
                            )
                    th = thp.tile([128, 1024], f16, tag="th")
                    nc.scalar.activation(
                        th[:], kp_ps[:], AF.Tanh,
                        bias=qpT_sb[:, gc * BL + b:gc * BL + b + 1],
                    )
                    tanhT[gc][v] = th
            state[b]["tanhT"] = tanhT

        def stage_vdot(b):
            tanhT = state[b]["tanhT"]
            sc_ps = sp.tile([128, 1024], f32, tag="sc")
            if b < 2:
                # pool has 2 rotating slots; rows other than 0/32/64/96 are
                # never written afterwards, so zeroing the first use of each
                # slot keeps the masked-sum garbage rows finite forever.
                nc.vector.memset(sc_ps[:], 0.0)
            # gc-outer so consecutive matmuls hit different column groups
            # (4-way tile_position packing) while keeping start before stop
            # within each psum region.
            for gc in range(2):
                for u in range(8):
                    r, c = u % 4, u // 4
                    nc.tensor.matmul(
                        sc_ps[32 * r:32 * r + 1, 512 * c:512 * (c + 1)],
                        v_sb[:, gc:gc + 1],
                        tanhT[gc][u // 2][:, 512 * (u % 2):512 * (u % 2 + 1)],
                        start=(gc == 0),
                        stop=(gc == 1),
                        tile_position=(0, 32 * r),
                    )
            state[b]["sc_ps"] = sc_ps

        def stage_softmaxA(b):
            sc_ps = state[b]["sc_ps"]
            # shifted softmax: exp(s - 8) keeps the fp16 staging copy in range;
            # the shift cancels in the normalization. (|scores| <= ||V||_1 ~ 13.)
            E = ep.tile([128, 1024], f32, tag="E")
            par = smp.tile([128, 1], f32, tag="par")
            nc.scalar.activation(E[:], sc_ps[:], AF.Exp, bias=negC[:, 0:1],
                                 accum_out=par[:, 0:1])
            # fp16 exp row (s-ordered) for the context reduction: ctx is
            # accumulated unnormalized and scaled by 1/sum at flush time, so
            # nothing here waits on the sum chain.
            st = stp.tile([1, S], f16, tag="st")
            for c in range(2):
                nc.gpsimd.dma_start(
                    st[0:1, 2048 * c:2048 * (c + 1)].rearrange(
                        "p (r f) -> p r f", r=4),
                    E[0:97:32, 512 * c:512 * (c + 1)],
                )
            wb = wbp.tile([128, S], f16, tag="wb")
            nc.gpsimd.partition_broadcast(wb[:], st[0:1, :])
            state[b]["wb"] = wb
            state[b]["E"] = E

            parm = smp.tile([128, 1], f32, tag="parm")
            nc.vector.tensor_scalar(
                out=parm[:], in0=par[:], scalar1=mask[:, 0:1], scalar2=None,
                op0=ALU.mult,
            )
            sumbc = smp.tile([128, 1], f32, tag="sumbc")
            nc.gpsimd.partition_all_reduce(
                sumbc[:], parm[:], channels=128, reduce_op=bass_isa.ReduceOp.add
            )
            state[b]["sumbc"] = sumbc

        def stage_softmaxB(b):
            # deferred by one batch: by now the partition all-reduce is done,
            # so the reciprocal doesn't stall the in-order DVE stream.
            E, sumbc = state[b]["E"], state[b]["sumbc"]
            recip = smp.tile([128, 1], f32, tag="recip")
            nc.vector.reciprocal(recip[:], sumbc[:])
            # recip rows for the flush-time context normalization ([8,1] tiles,
            # partition base 0 — DVE requires 32-aligned bases, DMA does not).
            half, row = b // 4, (2 * b) % 8
            nc.sync.dma_start(
                recipT[half][row:row + 2, 0:1], recip[row:row + 2, 0:1]
            )
            w32 = wp.tile([128, 1024], f32, tag="w32")
            nc.vector.tensor_scalar(
                out=w32[:], in0=E[:], scalar1=recip[:, 0:1], scalar2=None,
                op0=ALU.mult,
            )
            # weights out: s = 2048*c + 512*r + f  (r = row/32, c = col/512)
            for c in range(2):
                nc.sync.dma_start(
                    weights_d[b:b + 1, 2048 * c:2048 * (c + 1)].rearrange(
                        "b (r f) -> b r f", r=4),
                    w32[0:97:32, 512 * c:512 * (c + 1)],
                )

        def stage_ctx(b):
            kT, wb = state[b]["kT"], state[b]["wb"]
            # unnormalized context: ctx_raw[h] = sum_s exp16[s] * keysT[h, s]
            for hc in range(2):
                scratch = scp.tile([128, S], f16, tag="scratch")
                nc.vector.scalar_tensor_tensor(
                    out=scratch[:],
                    in0=kT[hc][:],
                    scalar=1.0,
                    in1=wb[:],
                    op0=ALU.mult,
                    op1=ALU.mult,
                    accum_out=ctx_all[:, 2 * b + hc:2 * b + hc + 1],
                )

        def flush_ctx(half):
            # transpose ctx_all[:, 8h:8h+8] -> [8, 128], normalize, store
            cols = ctx_all[:, 8 * half:8 * (half + 1)]
            ctxT_ps = sp.tile([8, 128], f32, tag="sc")
            nc.tensor.transpose(ctxT_ps[:], cols, ident[:])
            ctxT = const.tile([8, 128], f32, tag=f"ctxT{half}")
            nc.scalar.copy(ctxT[:], ctxT_ps[:])
            nc.vector.tensor_scalar(
                out=ctxT[:], in0=ctxT[:],
                scalar1=recipT[half][:, 0:1], scalar2=None,
                op0=ALU.mult,
            )
            nc.sync.dma_start(ctx_d[8 * half:8 * (half + 1), :], ctxT[:])

        stage_load(0)
        stage_load(1)
        stage_kp(0)
        for b in range(BL):
            if b + 1 < BL:
                stage_kp(b + 1)
            if b + 2 < BL:
                stage_load(b + 2)
            stage_vdot(b)
            stage_softmaxA(b)
            stage_ctx(b)
            if b >= 1:
                stage_softmaxB(b - 1)
            if b == BL - 3:
                flush_ctx(0)
        stage_softmaxB(BL - 1)
        flush_ctx(1)

    nc.compile()
    _prog_cache["nc"] = nc
    return nc


def _prep_inputs(query, keys, Wq, Wk, V):
    """Host-side marshalling: shard over batch, pre-transpose/cast."""
    query = np.asarray(query, dtype=np.float32)
    keys = np.asarray(keys, dtype=np.float32)
    Wq = np.asarray(Wq, dtype=np.float32)
    Wk = np.asarray(Wk, dtype=np.float32)
    V = np.asarray(V, dtype=np.float32)

    qp = query @ Wq.T  # [B, H] f32 (exact, tiny)
    wkT16 = np.ascontiguousarray(Wk.T).astype(np.float16)
    v16 = np.ascontiguousarray(V.reshape(2, 128).T).astype(np.float16)
    ident32 = np.eye(128, dtype=np.float32)
    keys16 = keys.astype(np.float16)

    in_maps = []
    for i in range(NCORES):
        sl = slice(BL * i, BL * (i + 1))
        keysT16 = np.ascontiguousarray(keys16[sl].transpose(0, 2, 1))  # [BL, H, S]
        # qpT_sb [128, (gc, b)]: qpT[p, gc*BL + b] = qp[b, gc*128 + p]
        qpT = np.ascontiguousarray(
            qp[sl].reshape(BL, 2, 128).transpose(2, 1, 0).reshape(128, 2 * BL)
        )
        in_maps.append({
            "keysT16": keysT16,
            "wkT16": wkT16,
            "qpT": qpT,
            "v16": v16,
            "ident32": ident32,
        })
    return in_maps


def run_device(query, keys, Wq, Wk, V, trace=False, trace_kwargs=None):
    from concourse.bass_utils import run_bass_kernel_spmd

    nc = _build_program()
    in_maps = _prep_inputs(query, keys, Wq, Wk, V)
    res = run_bass_kernel_spmd(
        nc, in_maps, list(range(NCORES)), trace=trace, **(trace_kwargs or {})
    )

    context = np.empty((B, H), dtype=np.float32)
    weights = np.empty((B, S), dtype=np.float32)
    for i in range(NCORES):
        r = res.results[i]
        sl = slice(BL * i, BL * (i + 1))
        weights[sl] = r["weights"]
        context[sl] = r["context"].reshape(BL, H)
    return (context, weights), res


def kernel(query, keys, Wq, Wk, V):
    (context, weights), _ = run_device(query, keys, Wq, Wk, V, trace=False)
    return (context, weights)


# revision 24
# speedup vs baseline: 1.1669x; 1.0296x over previous
"""Bahdanau attention on 8 Trainium2 NeuronCores (Bass/Tile), data-parallel over batch.

Problem shapes: query [64, 256] f32, keys [64, 4096, 256] f32, Wq/Wk [256, 256] f32,
V [256] f32.  Returns (context [64, 256] f32, weights [64, 4096] f32) matching

    q_proj = (query @ Wq.T)[:, None, :]
    k_proj = einsum('bsh,gh->bsg', keys, Wk)
    scores = einsum('bsh,h->bs', tanh(q_proj + k_proj), V)
    weights = softmax(scores, -1)
    context = einsum('bs,bsh->bh', weights, keys)

Sharding: batch 64 -> 8 per core; Wk/V replicated; q_proj (a [64,256] linear input
transform) is folded on the host into a per-core bias tensor.

Per-core device pipeline (B=8 local batches, S=4096, H=G=256):
  - keys are shipped pre-transposed / fp16 (keysT16 [8, 256, 4096]) so the big
    k_proj matmul can contract over h with h on partitions, and the context
    reduction can run on the Vector engine along the free (s) axis.
  - k_projT tiles [128 g, 1024 s] accumulate in PSUM over 2 h-chunks (fp16 inputs,
    fp32 accumulate); ScalarE applies tanh with the per-partition q_proj bias.
  - scores via TensorE dot with V as the 1-column stationary; each s-chunk of 512
    lands on psum row 32*(chunk%4) so one dense [128,1024] Exp (with accum_out)
    computes all 4096 exps + row sums; garbage rows are masked off and the total
    is spread to all partitions with a GpSimd partition all-reduce.
  - weights = E * (1/sum) on VectorE; DMA'd out with a strided AP that undoes the
    row-interleave.  A cast-DMA gathers the fp16 weights row, GpSimd broadcasts it
    to 128 partitions, and scalar_tensor_tensor (mult, mult, accum) reduces
    weights*keysT along s for the context.
"""

import os
import numpy as np
from contextlib import ExitStack

B, S, H = 64, 4096, 256
NCORES = 8
BL = B // NCORES  # local batches per core

_prog_cache = {}


def _build_program():
    if "nc" in _prog_cache:
        return _prog_cache["nc"]

    import concourse.bass as bass
    import concourse.tile as tile
    from concourse import bacc, mybir
    from concourse import bass_isa

    f32, f16 = mybir.dt.float32, mybir.dt.float16
    AF = mybir.ActivationFunctionType
    ALU = mybir.AluOpType

    nc = bacc.Bacc("TRN2", target_bir_lowering=False, debug=False)

    keysT_d = nc.dram_tensor("keysT16", [BL, H, S], f16, kind="ExternalInput").ap()
    wkT_d = nc.dram_tensor("wkT16", [H, H], f16, kind="ExternalInput").ap()
    qpT_d = nc.dram_tensor("qpT", [128, 2 * BL], f32, kind="ExternalInput").ap()
    v_d = nc.dram_tensor("v16", [128, 2], f16, kind="ExternalInput").ap()
    ident_d = nc.dram_tensor("ident32", [128, 128], f32, kind="ExternalInput").ap()
    weights_d = nc.dram_tensor("weights", [BL, S], f32, kind="ExternalOutput").ap()
    ctx_d = nc.dram_tensor("context", [2 * BL, 128], f32, kind="ExternalOutput").ap()

    with tile.TileContext(nc) as tc, ExitStack() as ctx:
        const = ctx.enter_context(tc.tile_pool(name="const", bufs=1))
        ktp = ctx.enter_context(tc.tile_pool(name="ktp", bufs=8))
        thp = ctx.enter_context(tc.tile_pool(name="thp", bufs=24))
        ep = ctx.enter_context(tc.tile_pool(name="ep", bufs=2))
        wp = ctx.enter_context(tc.tile_pool(name="wp", bufs=2))
        wbp = ctx.enter_context(tc.tile_pool(name="wbp", bufs=2))
        scp = ctx.enter_context(tc.tile_pool(name="scp", bufs=2))
        stp = ctx.enter_context(tc.tile_pool(name="stp", bufs=2))
        smp = ctx.enter_context(tc.tile_pool(name="smp", bufs=16))
        kpp = ctx.enter_context(tc.tile_pool(name="kpp", bufs=2, space="PSUM"))
        sp = ctx.enter_context(tc.tile_pool(name="sp", bufs=2, space="PSUM"))

        # constants / small inputs
        wkT_sb = const.tile([128, 512], f16)  # col = hc*256 + g ; h = hc*128 + p
        nc.sync.dma_start(
            wkT_sb[:].rearrange("p (hc g) -> p hc g", hc=2),
            wkT_d.rearrange("(hc p) g -> p hc g", hc=2),
        )
        qpT_sb = const.tile([128, 2 * BL], f32)  # col = gc*BL + b
        nc.sync.dma_start(qpT_sb[:], qpT_d[:, :])
        v_sb = const.tile([128, 2], f16)
        nc.sync.dma_start(v_sb[:], v_d[:, :])
        ident = const.tile([128, 128], f32)
        nc.sync.dma_start(ident[:], ident_d[:, :])
        mask = const.tile([128, 1], f32)
        nc.vector.memset(mask[:], 0.0)
        for r in range(4):
            nc.vector.memset(mask[32 * r:32 * r + 1, :], 1.0)
        negC = const.tile([128, 1], f32)
        nc.vector.memset(negC[:], -8.0)
        recipT0 = const.tile([8, 1], f32, tag="recipT0")
        recipT1 = const.tile([8, 1], f32, tag="recipT1")
        recipT = [recipT0, recipT1]
        ctx_all = const.tile([128, 2 * BL], f32)  # col = 2*b + hc

        # --- per-batch stage emitters; emission order is software-pipelined so
        # the static per-engine instruction streams interleave batch b's tail
        # with batch b+1's matmuls (keeps TensorE dense/warm).
        state = {}

        def stage_load(b):
            kT = []
            for hc in range(2):
                t = ktp.tile([128, S], f16, tag="kT")
                nc.sync.dma_start(t[:], keysT_d[b, 128 * hc:128 * (hc + 1), :])
                kT.append(t)
            state[b] = {"kT": kT}

        def stage_kp(b):
            kT = state[b]["kT"]
            tanhT = [[None] * 4 for _ in range(2)]
            for gc in range(2):
                for v in range(4):
                    kp_ps = kpp.tile([128, 1024], f32, tag="kp")
                    for hc in range(2):
                        for du in range(2):
                            u = 2 * v + du
                            nc.tensor.matmul(
                                kp_ps[:, 512 * du:512 * (du + 1)],
                                wkT_sb[:, 256 * hc + 128 * gc:256 * hc + 128 * (gc + 1)],
                                kT[hc][:, 512 * u:512 * (u + 1)],
                                start=(hc == 0),
                                stop=(hc == 1),
                            )
                    th = thp.tile([128, 1024], f16, tag="th")
                    nc.scalar.activation(
                        th[:], kp_ps[:], AF.Tanh,
                        bias=qpT_sb[:, gc * BL + b:gc * BL + b + 1],
                    )
                    tanhT[gc][v] = th
            state[b]["tanhT"] = tanhT

        def stage_vdot(b):
            tanhT = state[b]["tanhT"]
            sc_ps = sp.tile([128, 1024], f32, tag="sc")
            if b < 2:
                # pool has 2 rotating slots; rows other than 0/32/64/96 are
                # never written afterwards, so zeroing the first use of each
                # slot keeps the masked-sum garbage rows finite forever.
                nc.vector.memset(sc_ps[:], 0.0)
            # gc-outer so consecutive matmuls hit different column groups
            # (4-way tile_position packing) while keeping start before stop
            # within each psum region.
            for gc in range(2):
                for u in range(8):
                    r, c = u % 4, u // 4
                    nc.tensor.matmul(
                        sc_ps[32 * r:32 * r + 1, 512 * c:512 * (c + 1)],
                        v_sb[:, gc:gc + 1],
                        tanhT[gc][u // 2][:, 512 * (u % 2):512 * (u % 2 + 1)],
                        start=(gc == 0),
                        stop=(gc == 1),
                        tile_position=(0, 32 * r),
                    )
            state[b]["sc_ps"] = sc_ps

        def stage_softmaxA(b):
            sc_ps = state[b]["sc_ps"]
            # shifted softmax: exp(s - 8) keeps the fp16 staging copy in range;
            # the shift cancels in the normalization. (|scores| <= ||V||_1 ~ 13.)
            E = ep.tile([128, 1024], f32, tag="E")
            par = smp.tile([128, 1], f32, tag="par")
            nc.scalar.activation(E[:], sc_ps[:], AF.Exp, bias=negC[:, 0:1],
                                 accum_out=par[:, 0:1])
            # fp16 exp row (s-ordered) for the context reduction: ctx is
            # accumulated unnormalized and scaled by 1/sum at flush time, so
            # nothing here waits on the sum chain.
            st = stp.tile([1, S], f16, tag="st")
            for c in range(2):
                nc.gpsimd.dma_start(
                    st[0:1, 2048 * c:2048 * (c + 1)].rearrange(
                        "p (r f) -> p r f", r=4),
                    E[0:97:32, 512 * c:512 * (c + 1)],
                )
            wb = wbp.tile([128, S], f16, tag="wb")
            nc.gpsimd.partition_broadcast(wb[:], st[0:1, :])
            state[b]["wb"] = wb
            state[b]["E"] = E

            parm = smp.tile([128, 1], f32, tag="parm")
            nc.vector.tensor_scalar(
                out=parm[:], in0=par[:], scalar1=mask[:, 0:1], scalar2=None,
                op0=ALU.mult,
            )
            sumbc = smp.tile([128, 1], f32, tag="sumbc")
            nc.gpsimd.partition_all_reduce(
                sumbc[:], parm[:], channels=128, reduce_op=bass_isa.ReduceOp.add
            )
            state[b]["sumbc"] = sumbc

        def stage_softmaxB(b):
            # deferred by one batch: by now the partition all-reduce is done,
            # so the reciprocal doesn't stall the in-order DVE stream.
            E, sumbc = state[b]["E"], state[b]["sumbc"]
            recip = smp.tile([128, 1], f32, tag="recip")
            nc.vector.reciprocal(recip[:], sumbc[:])
            # recip rows for the flush-time context normalization ([8,1] tiles,
            # partition base 0 — DVE requires 32-aligned bases, DMA does not).
            half, row = b // 4, (2 * b) % 8
            nc.sync.dma_start(
                recipT[half][row:row + 2, 0:1], recip[row:row + 2, 0:1]
            )
            w32 = wp.tile([128, 1024], f32, tag="w32")
            nc.scalar.activation(w32[:], E[:], AF.Copy, scale=recip[:, 0:1])
            # weights out: s = 2048*c + 512*r + f  (r = row/32, c = col/512)
            for c in range(2):
                nc.sync.dma_start(
                    weights_d[b:b + 1, 2048 * c:2048 * (c + 1)].rearrange(
                        "b (r f) -> b r f", r=4),
                    w32[0:97:32, 512 * c:512 * (c + 1)],
                )

        def stage_ctx(b):
            kT, wb = state[b]["kT"], state[b]["wb"]
            # unnormalized context: ctx_raw[h] = sum_s exp16[s] * keysT[h, s]
            for hc in range(2):
                scratch = scp.tile([128, S], f16, tag="scratch")
                nc.vector.scalar_tensor_tensor(
                    out=scratch[:],
                    in0=kT[hc][:],
                    scalar=1.0,
                    in1=wb[:],
                    op0=ALU.mult,
                    op1=ALU.mult,
                    accum_out=ctx_all[:, 2 * b + hc:2 * b + hc + 1],
                )

        def flush_ctx(half):
            # transpose ctx_all[:, 8h:8h+8] -> [8, 128], normalize, store
            cols = ctx_all[:, 8 * half:8 * (half + 1)]
            ctxT_ps = sp.tile([8, 128], f32, tag="sc")
            nc.tensor.transpose(ctxT_ps[:], cols, ident[:])
            ctxT = const.tile([8, 128], f32, tag=f"ctxT{half}")
            nc.scalar.copy(ctxT[:], ctxT_ps[:])
            nc.vector.tensor_scalar(
                out=ctxT[:], in0=ctxT[:],
                scalar1=recipT[half][:, 0:1], scalar2=None,
                op0=ALU.mult,
            )
            nc.sync.dma_start(ctx_d[8 * half:8 * (half + 1), :], ctxT[:])

        stage_load(0)
        stage_load(1)
        stage_kp(0)
        for b in range(BL):
            if b + 1 < BL:
                stage_kp(b + 1)
            if b + 2 < BL:
                stage_load(b + 2)
            stage_vdot(b)
            stage_softmaxA(b)
            stage_ctx(b)
            if b >= 1:
                stage_softmaxB(b - 1)
            if b == BL - 3:
                flush_ctx(0)
        stage_softmaxB(BL - 1)
        flush_ctx(1)

    nc.compile()
    _prog_cache["nc"] = nc
    return nc


def _prep_inputs(query, keys, Wq, Wk, V):
    """Host-side marshalling: shard over batch, pre-transpose/cast."""
    query = np.asarray(query, dtype=np.float32)
    keys = np.asarray(keys, dtype=np.float32)
    Wq = np.asarray(Wq, dtype=np.float32)
    Wk = np.asarray(Wk, dtype=np.float32)
    V = np.asarray(V, dtype=np.float32)

    qp = query @ Wq.T  # [B, H] f32 (exact, tiny)
    wkT16 = np.ascontiguousarray(Wk.T).astype(np.float16)
    v16 = np.ascontiguousarray(V.reshape(2, 128).T).astype(np.float16)
    ident32 = np.eye(128, dtype=np.float32)
    keys16 = keys.astype(np.float16)

    in_maps = []
    for i in range(NCORES):
        sl = slice(BL * i, BL * (i + 1))
        keysT16 = np.ascontiguousarray(keys16[sl].transpose(0, 2, 1))  # [BL, H, S]
        # qpT_sb [128, (gc, b)]: qpT[p, gc*BL + b] = qp[b, gc*128 + p]
        qpT = np.ascontiguousarray(
            qp[sl].reshape(BL, 2, 128).transpose(2, 1, 0).reshape(128, 2 * BL)
        )
        in_maps.append({
            "keysT16": keysT16,
            "wkT16": wkT16,
            "qpT": qpT,
            "v16": v16,
            "ident32": ident32,
        })
    return in_maps


def run_device(query, keys, Wq, Wk, V, trace=False, trace_kwargs=None):
    from concourse.bass_utils import run_bass_kernel_spmd

    nc = _build_program()
    in_maps = _prep_inputs(query, keys, Wq, Wk, V)
    res = run_bass_kernel_spmd(
        nc, in_maps, list(range(NCORES)), trace=trace, **(trace_kwargs or {})
    )

    context = np.empty((B, H), dtype=np.float32)
    weights = np.empty((B, S), dtype=np.float32)
    for i in range(NCORES):
        r = res.results[i]
        sl = slice(BL * i, BL * (i + 1))
        weights[sl] = r["weights"]
        context[sl] = r["context"].reshape(BL, H)
    return (context, weights), res


def kernel(query, keys, Wq, Wk, V):
    (context, weights), _ = run_device(query, keys, Wq, Wk, V, trace=False)
    return (context, weights)
